# revision 38
# baseline (speedup 1.0000x reference)
"""Trainium2 Bass kernel for batched dense attention.

Reference computation (per batch b):
    q = query @ Wq + bq ; k = key @ Wk + bk ; v = value @ Wv + bv
    out = softmax(BETA * q k^T) v

Shapes: query/key/value [4, 2048, 1024], weights [1024, 1024], out [4, 2048, 1024].

Sharding: 8 cores = (batch b, seq half h). Each core computes out rows
[b, h*1024:(h+1)*1024, :] from its query shard [1024, 1024] plus the full
key/value of its batch (K/V projection duplicated across the 2 cores of a
batch; no collectives).

Core algorithm (all matmuls in float32r: ~1.5e-4 rel err, 4x fp32 speed):
  - queryT/keyT: PE-transpose raw inputs (fp32), round to f32r on the
    PSUM->SBUF copy.
  - qTr = (query @ Wq + bq)^T as [kd, q] ; kTr = (key @ Wk + bk)^T as [kd, k]
    via lhsT=W chunks (natural layout), rhs=transposed inputs; per-partition
    bias folded into the PSUM->SBUF copy.
  - S^T tiles [k, q] = lhsT(kTr).T @ rhs(qTr); exp(BETA*S^T) on ScalarE
    directly PSUM->SBUF as f32r (unnormalized probabilities pT).
  - row sums via PE: lhsT=pT slice, rhs=ones -> [q, 1] accumulated in PSUM.
  - out2 = pT.T @ value (value streamed from DRAM, contraction over k).
  - out = (out2 @ Wv) * (1/rowsum) + bv  -- normalization deferred to the
    end (linear), applied with a fused scalar_tensor_tensor on VectorE.
  - out2^T via PE transposes of o2 blocks (f32r, packs of 4 per PSUM bank).
"""
import ml_dtypes
import numpy as np

import concourse.bass as bass
import concourse.bacc as bacc
import concourse.tile as tile
from concourse import masks, mybir
from concourse.tile import add_dep_helper
from concourse.bass_utils import run_bass_kernel_spmd

B, S, D = 4, 2048, 1024
KD = 1024  # key_dim == value_dim == D
VD = 1024
BETA = 1.0 / float(np.sqrt(D))
N_CORES = 8
QS = S // 2  # per-core query rows (1024)

F32 = mybir.dt.float32
F32R = mybir.dt.float32r
BF16 = mybir.dt.bfloat16

C_D = D // 128     # 8 contraction chunks over D
G_KD = KD // 128   # 8 kd chunks
KT = S // 128      # 16 key tiles
QBLK = 512         # q-block size
NQB = QS // QBLK   # 4 q blocks
NQS = QBLK // 128  # 2 q slices per block


DEBUG_TAPS = False


def build_kernel():
    nc = bacc.Bacc("TRN2", target_bir_lowering=False, debug=False,
                   num_devices=N_CORES)

    q_sh = nc.dram_tensor("q_sh", [QS, D], F32, kind="ExternalInput").ap()
    key_b = nc.dram_tensor("key_b", [S, D], F32, kind="ExternalInput").ap()
    val_b = nc.dram_tensor("val_b", [S, D], F32, kind="ExternalInput").ap()
    Wq = nc.dram_tensor("Wq", [D, KD], F32, kind="ExternalInput").ap()
    Wk = nc.dram_tensor("Wk", [D, KD], F32, kind="ExternalInput").ap()
    Wv = nc.dram_tensor("Wv", [D, VD], F32, kind="ExternalInput").ap()
    bq = nc.dram_tensor("bq", [KD], F32, kind="ExternalInput").ap()
    bk = nc.dram_tensor("bk", [KD], F32, kind="ExternalInput").ap()
    bv = nc.dram_tensor("bv", [VD], F32, kind="ExternalInput").ap()
    out = nc.dram_tensor("out", [QS, VD], F32, kind="ExternalOutput").ap()
    taps = None
    if DEBUG_TAPS:
        taps = {
            "t_qTr": nc.dram_tensor("t_qTr", [128, G_KD * QS], F32,
                                    kind="ExternalOutput").ap(),
            "t_kTr": nc.dram_tensor("t_kTr", [128, G_KD * S], F32,
                                    kind="ExternalOutput").ap(),
            "t_pT": nc.dram_tensor("t_pT", [128, KT * QBLK], BF16,
                                   kind="ExternalOutput").ap(),
            "t_rs": nc.dram_tensor("t_rs", [128, 2 * NQS], F32,
                                   kind="ExternalOutput").ap(),
            "t_o2T": nc.dram_tensor("t_o2T", [128, C_D * QBLK], F32,
                                    kind="ExternalOutput").ap(),
        }

    with tile.TileContext(nc) as tc:
        _body(tc, q_sh, key_b, val_b, Wq, Wk, Wv, bq, bk, bv, out, taps)
    nc.compile()
    return nc


def _body(tc, q_sh, key_b, val_b, Wq, Wk, Wv, bq, bk, bv, out, taps=None):
    nc = tc.nc
    Exp = mybir.ActivationFunctionType.Exp
    mult = mybir.AluOpType.mult
    add = mybir.AluOpType.add

    # ---- consolidated persistent constants (two tiles: f32 / f32r) ------
    # constf cols: [0:8]=bqT, [8:16]=bkT, [16:16+VD]=bvb,
    #              [1040] ones col, row0 [1048:1048+VD]=bv staging
    const_pool = tc.alloc_tile_pool(name="const", bufs=1)
    constf = const_pool.tile([128, 1952], F32, name="constf")
    bqT = constf[:, 0:8]
    bkT = constf[:, 8:16]
    bvb = constf[:, 16:16 + VD]
    ones_f = constf[:, 1040:1042]
    onesrow_f = constf[0:1, 1041:1041 + 128]
    bv_f = constf[0:1, 16:16 + VD]
    rrec_all = constf[:, 1168:1168 + 2 * (QS // 128)]
    rs_sb = constf[0:2, 1200:1200 + 512]  # per-qs recip columns
    # constr cols: [0:128]=ident_r, row0 [136:136+VD]=bv_r,
    #              [1164:1164+128] onesrow_r
    constr = const_pool.tile([128, 1312], F32R, name="constr")
    ident_r = constr[:, 0:128]
    bv_r = constr[0:1, 136:136 + VD]
    onesrow_r = constr[0:1, 1164:1164 + 128]
    onesb_t = const_pool.tile([128, 2], BF16, name="onesb_t")
    onesb = onesb_t[:]

    for c in range(G_KD):
        nc.sync.dma_start(out=bqT[:, c:c + 1], in_=bq[c * 128:(c + 1) * 128])
        nc.sync.dma_start(out=bkT[:, c:c + 1], in_=bk[c * 128:(c + 1) * 128])
    nc.sync.dma_start(out=bv_f, in_=bv[:])
    nc.vector.memset(ones_f, 1.0)
    nc.vector.memset(onesrow_f, 1.0)
    nc.vector.tensor_copy(onesb, ones_f)
    nc.vector.tensor_copy(onesrow_r, onesrow_f)
    nc.vector.tensor_copy(bv_r, bv_f)

    # persistent big activations (allocated early: released late, LIFO)
    big_pool = tc.alloc_tile_pool(name="big", bufs=1)
    qTr = big_pool.tile([128, G_KD * QS], F32R, name="qTr")      # 32KB/p
    kTr = big_pool.tile([128, G_KD * S], F32R, name="kTr")       # 64KB/p

    ident_f = constf[:, 1824:1952]
    masks.make_identity(nc, ident_f)
    nc.vector.tensor_copy(ident_r, ident_f)

    psA = tc.alloc_tile_pool(name="psA", bufs=1, space="PSUM")

    # bv broadcast to all partitions via K=1 matmul
    for n in range(VD // 512):
        bc_ps = psA.tile([128, 512], F32, name="bc_ps", tag="mm", bufs=2)
        nc.tensor.matmul(bc_ps[:], onesrow_r,
                         bv_r[:, n * 512:(n + 1) * 512],
                         start=True, stop=True)
        nc.vector.tensor_copy(bvb[:, n * 512:(n + 1) * 512], bc_ps[:])

    # ===== P-K: key transpose + k projection (PE transposes) ==============
    HALF = S // 2
    wk_pool = tc.alloc_tile_pool(name="wk", bufs=1)
    Wkr = wk_pool.tile([128, C_D * KD], F32R, name="Wkr")
    for c in range(C_D):
        nc.gpsimd.dma_start(out=Wkr[:, c * KD:(c + 1) * KD],
                            in_=Wk[c * 128:(c + 1) * 128, :])

    kt_pool = tc.alloc_tile_pool(name="kt", bufs=1)
    n_krow = HALF // 128
    for kh in range(2):
        keyT = kt_pool.tile([128, C_D * HALF], F32R, name="keyT",
                            tag="keyT", bufs=1)
        for rt in range(n_krow):
            krow = kt_pool.tile([128, D], F32, name="krow", tag="krow", bufs=4)
            nc.sync.dma_start(
                out=krow[:],
                in_=key_b[kh * HALF + rt * 128:kh * HALF + (rt + 1) * 128, :])
            for cg in range(2):
                ktp_ps = psA.tile([128, 512], F32, name="ktp_ps", tag="tp",
                                  bufs=4)
                for j in range(4):
                    c = cg * 4 + j
                    nc.tensor.transpose(ktp_ps[:, j * 128:(j + 1) * 128],
                                        krow[:, c * 128:(c + 1) * 128],
                                        ident_f)
                nc.vector.tensor_copy(
                    keyT[:, rt * D + cg * 512:rt * D + (cg + 1) * 512],
                    ktp_ps[:])
        kT_v = keyT[:].rearrange("p (rt x) -> p rt x", rt=n_krow)
        for g in range(G_KD):
            for nt in range(HALF // 512):
                kmm_ps = psA.tile([128, 512], F32, name="kmm_ps", tag="mm",
                                  bufs=2)
                for c in range(C_D):
                    nc.tensor.matmul(
                        kmm_ps[:],
                        Wkr[:, c * KD + g * 128:c * KD + (g + 1) * 128],
                        kT_v[:, nt * 4:(nt + 1) * 4, c * 128:(c + 1) * 128],
                        start=(c == 0), stop=(c == C_D - 1))
                nc.vector.tensor_scalar(
                    out=kTr[:, g * S + kh * HALF + nt * 512:
                            g * S + kh * HALF + (nt + 1) * 512],
                    in0=kmm_ps[:], scalar1=bkT[:, g:g + 1], scalar2=None,
                    op0=add)
    kt_pool.release()
    wk_pool.release()

    # ===== P-Q: Wq load, query transpose, q projection ====================
    wq_pool = tc.alloc_tile_pool(name="wq", bufs=1)
    Wqr = wq_pool.tile([128, C_D * KD], F32R, name="Wqr")
    for c in range(C_D):
        nc.gpsimd.dma_start(out=Wqr[:, c * KD:(c + 1) * KD],
                            in_=Wq[c * 128:(c + 1) * 128, :])

    qt_pool = tc.alloc_tile_pool(name="qt", bufs=1)
    queryT = qt_pool.tile([128, C_D * QS], F32R, name="queryT")
    n_qrow = QS // 128
    for rt in range(n_qrow):
        qrow = qt_pool.tile([128, D], F32, name="qrow", tag="qrow", bufs=4)
        nc.sync.dma_start(out=qrow[:], in_=q_sh[rt * 128:(rt + 1) * 128, :])
        for cg in range(2):
            tp_ps = psA.tile([128, 512], F32, name="tp_ps", tag="tp", bufs=4)
            for j in range(4):
                c = cg * 4 + j
                nc.tensor.transpose(tp_ps[:, j * 128:(j + 1) * 128],
                                    qrow[:, c * 128:(c + 1) * 128], ident_f)
            nc.vector.tensor_copy(
                queryT[:, rt * D + cg * 512:rt * D + (cg + 1) * 512], tp_ps[:])

    qT_v = queryT[:].rearrange("p (rt x) -> p rt x", rt=n_qrow)
    for g in range(G_KD):
        for nt in range(QS // 512):
            mm_ps = psA.tile([128, 512], F32, name="mm_ps", tag="mm", bufs=2)
            for c in range(C_D):
                nc.tensor.matmul(
                    mm_ps[:],
                    Wqr[:, c * KD + g * 128:c * KD + (g + 1) * 128],
                    qT_v[:, nt * 4:(nt + 1) * 4, c * 128:(c + 1) * 128],
                    start=(c == 0), stop=(c == C_D - 1))
            nc.vector.tensor_scalar(
                out=qTr[:, g * QS + nt * 512:g * QS + (nt + 1) * 512],
                in0=mm_ps[:], scalar1=bqT[:, g:g + 1], scalar2=None, op0=add)
    qt_pool.release()
    wq_pool.release()
    psA.release()

    # ===== P6: attention main loop ========================================
    # All PSUM and SBUF working tiles are created ONCE and reused via
    # same-tile WAR dependencies (manual rotation). Dynamic pool-slot
    # handoff between independent chains can deadlock the Tile scheduler
    # (in-order engines + slot-wait cycles), so P6 avoids it entirely.
    # PSUM: sT(2) + rs(1) + o2(4) + op(1) = 8 banks.
    psB = tc.alloc_tile_pool(name="psB", bufs=1, space="PSUM")
    sT_tiles = [psB.tile([128, QBLK], F32, name=f"sT{i}", tag=f"sT{i}")
                for i in range(2)]
    rs_ps = psB.tile([128, 512], F32, name="rs_ps", tag="rs")
    o2_tiles = [psB.tile([128, 512], F32, name=f"o2_{i}", tag=f"o2_{i}")
                for i in range(NQS)]
    op_ps = psB.tile([128, 512], F32R, name="op_ps", tag="opb")
    op_f32 = op_ps[:].bitcast(F32)

    mn_pool = tc.alloc_tile_pool(name="mn", bufs=1)
    pT_tiles = [mn_pool.tile([128, KT * QBLK], BF16, name=f"pT{i}",
                             tag=f"pT{i}") for i in range(2)]
    o2T = mn_pool.tile([128, C_D * QBLK], F32R, name="o2T")
    o2r_all = mn_pool.tile([128, NQS * 512], F32R, name="o2r_all")
    o2r_tiles = [o2r_all[:, i * 512:(i + 1) * 512] for i in range(NQS)]
    vch = mn_pool.tile([128, 7 * 512], BF16, name="vch")  # 7-slice value ring
    ost_all = mn_pool.tile([128, 2 * 512], F32, name="ost_all")
    ostage_tiles = [ost_all[:, i * 512:(i + 1) * 512] for i in range(2)]
    Wvh_tiles = [mn_pool.tile([128, C_D * 512], F32R, name=f"Wvh{i}",
                              tag=f"Wvh{i}") for i in range(2)]
    for vd in range(2):
        for c in range(C_D):
            nc.gpsimd.dma_start(
                out=Wvh_tiles[vd][:, c * 512:(c + 1) * 512],
                in_=Wv[c * 128:(c + 1) * 128, vd * 512:(vd + 1) * 512])

    for qb in range(NQB):
        q0 = qb * QBLK
        pT = pT_tiles[qb % 2]
        # ---- phase A: S^T -> exp -> pT ; rowsums ----
        for kt in range(KT):
            sT_ps = sT_tiles[kt % 2]
            for g in range(G_KD):
                nc.tensor.matmul(
                    sT_ps[:],
                    kTr[:, g * S + kt * 128:g * S + (kt + 1) * 128],
                    qTr[:, g * QS + q0:g * QS + q0 + QBLK],
                    start=(g == 0), stop=(g == G_KD - 1))
            nc.scalar.activation(pT[:, kt * QBLK:(kt + 1) * QBLK], sT_ps[:],
                                 Exp, scale=float(BETA))
            # rowsums: ones.T @ pT -> [2, q 512], accumulated over kt
            nc.tensor.matmul(
                rs_ps[0:2, :],
                onesb,
                pT[:, kt * QBLK:(kt + 1) * QBLK],
                start=(kt == 0), stop=(kt == KT - 1),
                skip_group_check=True)
        rrec = rrec_all[:, qb * 2 * NQS:(qb + 1) * 2 * NQS]
        # stage [2,512] sums to SBUF, transpose back into per-partition
        # [128, 2] columns of the same (now-free) rs bank
        nc.vector.tensor_copy(rs_sb, rs_ps[0:2, :])
        for qs in range(NQS):
            nc.tensor.transpose(rs_ps[:, 2 * qs:2 * qs + 2],
                                rs_sb[:, qs * 128:(qs + 1) * 128],
                                ident_f[0:2, 0:2])
        nc.vector.reciprocal(rrec, rs_ps[:, 0:2 * NQS])

        # ---- phase B: out2 = pT.T @ value (vd-outer, value streamed) ----
        for vd in range(2):
            for kt in range(KT):
                vs = (kt % 7) * 512
                nc.gpsimd.dma_start(
                    out=vch[:, vs:vs + 512],
                    in_=val_b[kt * 128:(kt + 1) * 128,
                              vd * 512:(vd + 1) * 512])
                for qs in range(NQS):
                    nc.tensor.matmul(
                        o2_tiles[qs][:],
                        pT[:, kt * QBLK + qs * 128:kt * QBLK + (qs + 1) * 128],
                        vch[:, vs:vs + 512],
                        start=(kt == 0), stop=(kt == KT - 1))
            # free the o2 PSUM tiles first (copies with no PE deps), THEN
            # transpose+scatter
            for qs in range(NQS):
                nc.vector.tensor_copy(o2r_tiles[qs][:], o2_tiles[qs][:])
            for qs in range(NQS):
                o2r = o2r_tiles[qs]
                for u in range(4):
                    nc.tensor.transpose(op_ps[:, u * 128:(u + 1) * 128],
                                        o2r[:, u * 128:(u + 1) * 128],
                                        ident_r)
                # o2T[:, (vd*4+u)*QBLK + qs*128 : +128] <- op_ps[:, u*128:+128]
                src_ap = op_ps[:].rearrange("p (u f) -> p u f", u=4)
                dst = o2T[:].rearrange("p (c f) -> p c f", c=C_D)[
                    :, vd * 4:(vd + 1) * 4, qs * 128:(qs + 1) * 128]
                nc.vector.tensor_copy(dst, src_ap)

        if taps is not None and qb == 0:
            nc.sync.dma_start(out=taps["t_o2T"][:], in_=o2T[:].bitcast(F32))

        # ---- phase C: out = (out2 @ Wv) * rrec + bv ----
        for vd in range(2):
            Wvh = Wvh_tiles[vd]
            for qs in range(NQS):
                for c in range(C_D):
                    nc.tensor.matmul(
                        op_f32,
                        o2T[:, c * QBLK + qs * 128:c * QBLK + (qs + 1) * 128],
                        Wvh[:, c * 512:(c + 1) * 512],
                        start=(c == 0), stop=(c == C_D - 1))
                ostage = ostage_tiles[qs % 2]
                nc.vector.scalar_tensor_tensor(
                    out=ostage[:], in0=op_f32, scalar=rrec[:, 2 * qs:2 * qs + 1],
                    in1=bvb[:, vd * 512:(vd + 1) * 512], op0=mult, op1=add)
                nc.scalar.dma_start(
                    out=out[q0 + qs * 128:q0 + (qs + 1) * 128,
                            vd * 512:(vd + 1) * 512],
                    in_=ostage[:])

    mn_pool.release()
    psB.release()
    big_pool.release()
    const_pool.release()


_NC_CACHE = {}


def _get_nc():
    if "nc" not in _NC_CACHE:
        _NC_CACHE["nc"] = build_kernel()
    return _NC_CACHE["nc"]


def kernel(query, key, value, Wq, bq, Wk, bk, Wv, bv):
    query = np.ascontiguousarray(np.asarray(query, dtype=np.float32))
    key = np.ascontiguousarray(np.asarray(key, dtype=np.float32))
    value = np.ascontiguousarray(np.asarray(value, dtype=np.float32))
    Wq = np.ascontiguousarray(np.asarray(Wq, dtype=np.float32))
    Wk = np.ascontiguousarray(np.asarray(Wk, dtype=np.float32))
    Wv = np.ascontiguousarray(np.asarray(Wv, dtype=np.float32))
    bq = np.ascontiguousarray(np.asarray(bq, dtype=np.float32))
    bk = np.ascontiguousarray(np.asarray(bk, dtype=np.float32))
    bv = np.ascontiguousarray(np.asarray(bv, dtype=np.float32))

    nc = _get_nc()
    in_maps = make_in_maps(query, key, value, Wq, bq, Wk, bk, Wv, bv)
    res = run_bass_kernel_spmd(nc, in_maps, list(range(N_CORES)))
    outp = np.empty((B, S, VD), dtype=np.float32)
    for core in range(N_CORES):
        b, h = divmod(core, 2)
        outp[b, h * QS:(h + 1) * QS, :] = res.results[core]["out"]
    return outp


def make_in_maps(query, key, value, Wq, bq, Wk, bk, Wv, bv):
    in_maps = []
    for core in range(N_CORES):
        b, h = divmod(core, 2)
        in_maps.append({
            "q_sh": np.ascontiguousarray(query[b, h * QS:(h + 1) * QS, :]),
            "key_b": key[b],
            "val_b": value[b],
            "Wq": Wq, "Wk": Wk, "Wv": Wv,
            "bq": bq, "bk": bk, "bv": bv,
        })
    return in_maps
